# revision 18
# baseline (speedup 1.0000x reference)
"""Bass/Trainium2 kernel for nn_EquivariantProductBasisBlock.

Math (per node n, feature f):
    s = x[n,f,0]; v = x[n,f,1:4]; vv = (v.v)/sqrt(3)
    out0 = a0[sp,0]*s + a0[sp,1]*s^2 + a0[sp,2]*vv + a0[sp,3]*s^3 + a0[sp,4]*(s*vv)
    c1   = a1[sp,0] + a1[sp,1]*s + a1[sp,2]*s^2 + a1[sp,3]*vv
    y0 = out0 @ W0 / 16 ;  y1_c = (c1*v_c) @ W1 / 16
    out = concat(y0, y1) over the lm axis.

Strategy: shard nodes over 8 cores. Host sorts nodes by species so the
species-dependent path weights become per-partition scalar columns on
device (features on partitions, nodes on the free axis). The 1/sqrt(3)
and 1/16 factors are folded into the weight tables on the host.
Elementwise + GEMM run in bf16 (fp32 PSUM accumulation); I/O is bf16
with the final output upcast on the host.

Engine assignment (v3): DVE is the critical path. GPSIMD elementwise
offload was tried and reverted: its SBUF port is shared with the DVE
and concurrent streaming inflated DVE op costs ~3-5x. Instead the
h4 = h3 + a00 / out0 = s*h4 pair is fused into one
scalar_tensor_tensor (A0 = (h3 + a00) * s) and the w affine joins h2
on the ScalarE to balance DVE vs ACT.

Polynomial factorization (per-species coefficients fold into
ScalarE affines, tensor_scalar per-partition scalars, or the STT
scalar operand):
    h2 = a3*s + a1           (ACT affine, per species)
    w  = a13*vv + a10        (ACT affine)
    B  = a4*s + a2           (DVE tensor_scalar)
    gg = a12*s + a11         (DVE tensor_scalar)
    [h3, p1] = s * [h2, gg]  (DVE pair tensor_tensor)
    A0 = (h3 + a0) * s       (DVE scalar_tensor_tensor)
    T  = vv * B              (DVE tensor_tensor)
    out0 = A0 + T            (PE PSUM accumulation, two movings)
    c1 = p1 + w              (DVE tensor_tensor)
    rhs_c = c1 * v_c         (DVE tensor_tensor, c1 broadcast x3)
"""

import numpy as np
from contextlib import ExitStack

import ml_dtypes

N_CORES = 8
F = 256
NUM_SPECIES = 10
NB = 512    # nodes per compute block
SUB = 512   # nodes per matmul / store sub-block (one PSUM bank fp32)
INV_SQRT3 = 1.0 / np.sqrt(3.0)
INV_SQRT_F = 1.0 / np.sqrt(256.0)

_KERNEL_CACHE = {}


def _build_bass(c_sp, ntot):
    """Build + compile the per-core Bass graph.

    c_sp: per-species padded segment length (same on every core), even.
    ntot: total padded nodes per core (multiple of SUB).
    """
    import concourse.bacc as bacc
    import concourse.bass as bass
    import concourse.mybir as mybir
    import concourse.tile as tile

    fp32 = mybir.dt.float32
    bf16 = mybir.dt.bfloat16
    AF = mybir.ActivationFunctionType
    OP = mybir.AluOpType

    nc = bacc.Bacc("TRN2", target_bir_lowering=False, debug=False)

    x = nc.dram_tensor("x", [10, 128, ntot], bf16, kind="ExternalInput")
    a0 = nc.dram_tensor("a0", [256, 5 * NUM_SPECIES], fp32, kind="ExternalInput")
    a1 = nc.dram_tensor("a1", [256, 4 * NUM_SPECIES], fp32, kind="ExternalInput")
    w0 = nc.dram_tensor("w0", [256, 256], bf16, kind="ExternalInput")
    w1 = nc.dram_tensor("w1", [256, 256], bf16, kind="ExternalInput")
    y = nc.dram_tensor("y", [8, 128, ntot], bf16, kind="ExternalOutput")

    xr = x[:].rearrange("s p n -> p s n")
    yr = y[:].rearrange("s p n -> p s n")

    # node blocks (small first blocks for pipeline ramp-in)
    blocks = [(0, 256), (256, 256)]
    j = 512
    while j < ntot:
        nb = min(NB, ntot - j)
        blocks.append((j, nb))
        j += nb

    ends = np.cumsum(c_sp)

    def segments(j0, nb):
        segs = []
        for sp in range(NUM_SPECIES):
            lo = int(ends[sp] - c_sp[sp])
            hi = int(ends[sp])
            a = max(lo, j0)
            b = min(hi, j0 + nb)
            if a < b:
                segs.append((sp, a - j0, b - a))
        return segs

    with tile.TileContext(nc) as tc:
        with ExitStack() as ctx:
            consts = ctx.enter_context(tc.tile_pool(name="consts", bufs=1))
            io_in = ctx.enter_context(tc.tile_pool(name="io_in", bufs=4))
            rhs_p = ctx.enter_context(tc.tile_pool(name="rhs", bufs=2))
            tmp = ctx.enter_context(tc.tile_pool(name="tmp", bufs=2))
            stag = ctx.enter_context(tc.tile_pool(name="stag", bufs=2))
            psum = ctx.enter_context(tc.tile_pool(name="psum", bufs=2, space="PSUM"))

            # --- constants (DMAs issued after the first input block below) ---
            w0_sb = consts.tile([128, 2, 256], bf16)
            w1_sb = consts.tile([128, 2, 256], bf16)
            a0_sb = consts.tile([128, 2, 5 * NUM_SPECIES], fp32)
            a1_sb = consts.tile([128, 2, 4 * NUM_SPECIES], fp32)
            consts_emitted = [False]

            def emit_const_dmas():
                nc.sync.dma_start(out=a0_sb, in_=a0[:].rearrange("(fc p) c -> p fc c", p=128))
                nc.sync.dma_start(out=a1_sb, in_=a1[:].rearrange("(fc p) c -> p fc c", p=128))
                nc.sync.dma_start(out=w0_sb, in_=w0[:].rearrange("(fc p) g -> p fc g", p=128))
                nc.sync.dma_start(out=w1_sb, in_=w1[:].rearrange("(fc p) g -> p fc g", p=128))
                consts_emitted[0] = True

            def a0c(fc, sp, p):
                i = sp * 5 + p
                return a0_sb[:, fc, i : i + 1]

            def a1c(fc, sp, p):
                i = sp * 4 + p
                return a1_sb[:, fc, i : i + 1]

            def flush(pend):
                """Deferred PSUM->SBUF copies + output DMA for a finished block.

                Emitted one iteration late so the ACT queue sees the next
                block's affines before these copies (no head-of-line block)
                and the output DMA (SWDGE/gpsimd) never delays input DMA
                triggers on the Sync queue.
                """
                pj0, pnb, pairs = pend
                stg = stag.tile([128, 8, pnb], bf16, tag="stg", name=f"stg_{pj0}")
                for pi in range(2):
                    nc.scalar.activation(
                        stg[:, pi * 4 : pi * 4 + 4, :],
                        pairs[pi][:, :, :, :pnb].rearrange("p c g n -> p (c g) n"),
                        AF.Copy,
                    )
                nc.gpsimd.dma_start(out=yr[:, :, pj0 : pj0 + pnb], in_=stg)

            pending = None
            for (j0, nb) in blocks:
                segs = segments(j0, nb)

                xin = io_in.tile([128, 5, 2, nb], bf16, tag="xin", name=f"xin_{j0}")
                nc.sync.dma_start(
                    out=xin.rearrange("p c f n -> p (c f) n"),
                    in_=xr[:, :, j0 : j0 + nb],
                )
                if not consts_emitted[0]:
                    emit_const_dmas()

                rhs = rhs_p.tile([128, 3, 2, nb], bf16, tag="rhs", name=f"rhs_{j0}")

                v_all = xin[:, 0:3, :, :]   # [128, 3, 2, nb]
                s_all = xin[:, 3, :, :]     # [128, 2, nb]
                vv = xin[:, 4, :, :]        # [128, 2, nb] host-precomputed v.v
                sv_pair = xin[:, 3:5, :, :]  # [s, vv] adjacent channels

                hg = tmp.tile([128, 2, 2, nb], bf16, tag="hg", bufs=2)
                h2 = hg[:, 0]
                gg = hg[:, 1]
                hb = tmp.tile([128, 2, 2, nb], bf16, tag="hb", bufs=2)
                h4 = hb[:, 0]
                bb = hb[:, 1]
                w_ = tmp.tile([128, 2, nb], bf16, tag="w", bufs=2)
                hp = tmp.tile([128, 2, 2, nb], bf16, tag="hp", bufs=2)
                h3 = hp[:, 0]
                p1 = hp[:, 1]
                at = tmp.tile([128, 2, 2, nb], bf16, tag="at", bufs=2)
                c1 = tmp.tile([128, 2, nb], bf16, tag="c1", bufs=2)

                # per-species affines: h2/w0 on ScalarE; bb/gg/w1 as DVE tensor_scalar
                for fc in range(2):
                    for (sp, o, L) in segs:
                        sl = slice(o, o + L)
                        nc.vector.tensor_scalar(
                            bb[:, fc, sl], s_all[:, fc, sl],
                            a0c(fc, sp, 4), a0c(fc, sp, 2), OP.mult, OP.add,
                        )
                        nc.scalar.activation(h2[:, fc, sl], s_all[:, fc, sl], AF.Identity,
                                             bias=a0c(fc, sp, 1), scale=a0c(fc, sp, 3))
                        nc.vector.tensor_scalar(
                            gg[:, fc, sl], s_all[:, fc, sl],
                            a1c(fc, sp, 2), a1c(fc, sp, 1), OP.mult, OP.add,
                        )
                        if fc == 0:  # split w across ACT/DVE to balance the queues
                            nc.scalar.activation(w_[:, fc, sl], vv[:, fc, sl], AF.Identity,
                                                 bias=a1c(fc, sp, 0), scale=a1c(fc, sp, 3))
                        else:
                            nc.vector.tensor_scalar(
                                w_[:, fc, sl], vv[:, fc, sl],
                                a1c(fc, sp, 3), a1c(fc, sp, 0), OP.mult, OP.add,
                            )

                # h3 = s*h2 ; p1 = s*gg  (one op, s broadcast over the pair)
                s_b2 = bass.AP(
                    tensor=xin.tensor,
                    offset=s_all.offset,
                    ap=[s_all.ap[0], [0, 2], s_all.ap[1], s_all.ap[2]],
                )
                nc.vector.tensor_tensor(hp, s_b2, hg, OP.mult)

                # h4 = h3 + a00 (single-op TS, 4x)
                for fc in range(2):
                    for (sp, o, L) in segs:
                        sl = slice(o, o + L)
                        nc.vector.tensor_scalar(
                            h4[:, fc, sl], h3[:, fc, sl], a0c(fc, sp, 0), None, OP.add,
                        )

                # [A0, T] = [s, vv] * [h4, bb]  (one paired op)
                nc.vector.tensor_tensor(at, sv_pair, hb, OP.mult)

                # c1 = p1 + w
                nc.vector.tensor_tensor(c1, p1, w_, OP.add)

                # rhs_c = c1 * v_c  (c1 broadcast over the 3 components)
                c1b = bass.AP(
                    tensor=c1.tensor,
                    offset=c1.offset,
                    ap=[c1.ap[0], [0, 3], c1.ap[1], c1.ap[2]],
                )
                nc.vector.tensor_tensor(rhs, c1b, v_all, OP.mult)

                # deferred stores of the PREVIOUS block (ACT runs them after
                # this block's affines; PE of this block overlaps them)
                if pending is not None:
                    flush(pending)

                # --- GEMM: y[comp] = rhs[comp] @ W (K=256 over fc chunks) ---
                # PSUM tiles hold comp pairs (01, 23) so stores pair up.
                # full-SUB stride keeps each [*, c, g, :] slice bank-aligned
                # (a ragged nb would otherwise make matmul dests cross banks)
                pairs = []
                for pi in range(2):
                    pairs.append(
                        psum.tile([128, 2, 2, SUB], fp32, tag="ps", name=f"ps{pi}_{j0}")
                    )
                for comp in range(4):
                    pst = pairs[comp // 2]
                    w_sb = w0_sb if comp == 0 else w1_sb
                    for gc in range(2):
                        g0 = gc * 128
                        for fc in range(2):
                            lhsT = w_sb[:, fc, g0 : g0 + 128]
                            movings = (
                                [at[:, 0, fc], at[:, 1, fc]]
                                if comp == 0
                                else [rhs[:, comp - 1, fc]]
                            )
                            for mi, mv in enumerate(movings):
                                nc.tensor.matmul(
                                    pst[:, comp % 2, gc, :nb],
                                    lhsT,
                                    mv,
                                    start=(fc == 0 and mi == 0),
                                    stop=(fc == 1 and mi == len(movings) - 1),
                                )
                pending = (j0, nb, pairs)

            flush(pending)

    nc.compile()
    return nc


def _prepare(node_feats, node_specie, w0, w1, W0, W1):
    """Host-side: sort by species, shard, transpose, fold scale factors."""
    n = node_feats.shape[0]
    sp = np.asarray(node_specie).astype(np.int64)

    ids_by_sp = [np.nonzero(sp == s)[0] for s in range(NUM_SPECIES)]
    # near-equal contiguous chunks per core (max-min <= 1) minimize the
    # per-species padded length (= max over cores)
    chunks = [np.array_split(ids_by_sp[s], N_CORES) for s in range(NUM_SPECIES)]
    core_ids = [[chunks[s][c] for s in range(NUM_SPECIES)] for c in range(N_CORES)]
    # even segment lengths keep bf16 slices 4B-aligned on device
    c_sp = [
        (max(len(core_ids[c][s]) for c in range(N_CORES)) + 1) // 2 * 2
        for s in range(NUM_SPECIES)
    ]
    ntot = int(np.sum(c_sp))  # even; tail block may be ragged (< SUB)

    idx = np.zeros((N_CORES, ntot), dtype=np.int64)
    valid = np.zeros((N_CORES, ntot), dtype=bool)
    off = 0
    for s in range(NUM_SPECIES):
        L = c_sp[s]
        for c in range(N_CORES):
            ids = core_ids[c][s]
            k = len(ids)
            idx[c, off : off + k] = ids
            valid[c, off : off + k] = True
        off += L

    w0a = np.asarray(w0, np.float32).copy()
    w1a = np.asarray(w1, np.float32).copy()
    w0a[:, 2, :] *= INV_SQRT3
    w0a[:, 4, :] *= INV_SQRT3
    w1a[:, 3, :] *= INV_SQRT3
    a0_tab = np.ascontiguousarray(
        w0a.transpose(2, 0, 1).reshape(F, 5 * NUM_SPECIES)
    )
    a1_tab = np.ascontiguousarray(
        w1a.transpose(2, 0, 1).reshape(F, 4 * NUM_SPECIES)
    )
    W0s = (np.asarray(W0, np.float32) * INV_SQRT_F).astype(ml_dtypes.bfloat16)
    W1s = (np.asarray(W1, np.float32) * INV_SQRT_F).astype(ml_dtypes.bfloat16)

    nf = np.asarray(node_feats, np.float32)
    n_ = nf.shape[0]
    # channel order [v1, v2, v3, s, vv]: s/vv adjacent for the paired
    # [A0, T] = [s, vv] * [h4, bb] tensor_tensor on device
    xf = np.empty((5, F, n_), np.float32)
    xf[0:3] = nf[:, :, 1:4].transpose(2, 1, 0)
    xf[3] = nf[:, :, 0].T
    xf[4] = (nf[:, :, 1] ** 2 + nf[:, :, 2] ** 2 + nf[:, :, 3] ** 2).T
    xt = xf.astype(ml_dtypes.bfloat16)  # [5,256,n]
    xs = []
    for c in range(N_CORES):
        xc = xt[:, :, idx[c]]
        xs.append(np.ascontiguousarray(xc.reshape(10, 128, ntot)))

    return xs, idx, valid, tuple(c_sp), ntot, a0_tab, a1_tab, W0s, W1s


def kernel(node_feats, node_specie, w0, w1, W0, W1):
    from concourse.bass_utils import run_bass_kernel_spmd

    xs, idx, valid, c_sp, ntot, a0_tab, a1_tab, W0s, W1s = _prepare(
        node_feats, node_specie, w0, w1, W0, W1
    )

    key = (c_sp, ntot)
    if key not in _KERNEL_CACHE:
        _KERNEL_CACHE[key] = _build_bass(list(c_sp), ntot)
    nc = _KERNEL_CACHE[key]

    in_maps = [
        {"x": xs[c], "a0": a0_tab, "a1": a1_tab, "w0": W0s, "w1": W1s}
        for c in range(N_CORES)
    ]
    res = run_bass_kernel_spmd(nc, in_maps, core_ids=list(range(N_CORES)))

    n = node_feats.shape[0]
    out = np.empty((n, F, 4), dtype=np.float32)
    for c in range(N_CORES):
        yc = res.results[c]["y"].reshape(4, F, ntot).astype(np.float32)
        yt = np.ascontiguousarray(yc.transpose(2, 1, 0))
        m = valid[c]
        out[idx[c][m]] = yt[m]
    return out


# revision 23
# speedup vs baseline: 1.2274x; 1.2274x over previous
"""Bass/Trainium2 kernel for nn_EquivariantProductBasisBlock.

Math (per node n, feature f):
    s = x[n,f,0]; v = x[n,f,1:4]; vv = (v.v)/sqrt(3)
    out0 = a0[sp,0]*s + a0[sp,1]*s^2 + a0[sp,2]*vv + a0[sp,3]*s^3 + a0[sp,4]*(s*vv)
    c1   = a1[sp,0] + a1[sp,1]*s + a1[sp,2]*s^2 + a1[sp,3]*vv
    y0 = out0 @ W0 / 16 ;  y1_c = (c1*v_c) @ W1 / 16
    out = concat(y0, y1) over the lm axis.

Strategy: shard nodes over 8 cores. Host sorts nodes by species so the
species-dependent path weights become per-partition scalar columns on
device (features on partitions, nodes on the free axis). The 1/sqrt(3)
and 1/16 factors are folded into the weight tables on the host.
Elementwise + GEMM run in bf16 (fp32 PSUM accumulation); I/O is bf16
with the final output upcast on the host.

Engine assignment (v3): DVE is the critical path. GPSIMD elementwise
offload was tried and reverted: its SBUF port is shared with the DVE
and concurrent streaming inflated DVE op costs ~3-5x. Instead the
h4 = h3 + a00 / out0 = s*h4 pair is fused into one
scalar_tensor_tensor (A0 = (h3 + a00) * s) and the w affine joins h2
on the ScalarE to balance DVE vs ACT.

Polynomial factorization (per-species coefficients fold into
ScalarE affines, tensor_scalar per-partition scalars, or the STT
scalar operand):
    h2 = a3*s + a1           (ACT affine, per species)
    w  = a13*vv + a10        (ACT affine)
    B  = a4*s + a2           (DVE tensor_scalar)
    gg = a12*s + a11         (DVE tensor_scalar)
    [h3, p1] = s * [h2, gg]  (DVE pair tensor_tensor)
    A0 = (h3 + a0) * s       (DVE scalar_tensor_tensor)
    T  = vv * B              (DVE tensor_tensor)
    out0 = A0 + T            (PE PSUM accumulation, two movings)
    c1 = p1 + w              (DVE tensor_tensor)
    rhs_c = c1 * v_c         (DVE tensor_tensor, c1 broadcast x3)
"""

import numpy as np
from contextlib import ExitStack

import ml_dtypes

N_CORES = 8
F = 256
NUM_SPECIES = 10
NB = 512    # nodes per compute block
SUB = 512   # nodes per matmul / store sub-block (one PSUM bank fp32)
INV_SQRT3 = 1.0 / np.sqrt(3.0)
INV_SQRT_F = 1.0 / np.sqrt(256.0)

_KERNEL_CACHE = {}


def _build_bass(c_sp, ntot):
    """Build + compile the per-core Bass graph.

    c_sp: per-species padded segment length (same on every core), even.
    ntot: total padded nodes per core (multiple of SUB).
    """
    import concourse.bacc as bacc
    import concourse.bass as bass
    import concourse.mybir as mybir
    import concourse.tile as tile

    fp32 = mybir.dt.float32
    bf16 = mybir.dt.bfloat16
    AF = mybir.ActivationFunctionType
    OP = mybir.AluOpType

    nc = bacc.Bacc("TRN2", target_bir_lowering=False, debug=False)

    x = nc.dram_tensor("x", [10, 128, ntot], bf16, kind="ExternalInput")
    a0 = nc.dram_tensor("a0", [256, 5 * NUM_SPECIES], fp32, kind="ExternalInput")
    a1 = nc.dram_tensor("a1", [256, 4 * NUM_SPECIES], fp32, kind="ExternalInput")
    w0 = nc.dram_tensor("w0", [256, 256], bf16, kind="ExternalInput")
    w1 = nc.dram_tensor("w1", [256, 256], bf16, kind="ExternalInput")
    y = nc.dram_tensor("y", [8, 128, ntot], bf16, kind="ExternalOutput")

    xr = x[:].rearrange("s p n -> p s n")
    yr = y[:].rearrange("s p n -> p s n")

    # macro blocks (one x/y DMA each) of two compute sub-blocks
    # first macro is small for pipeline ramp-in
    macros = [(0, 512, [256, 256])]
    j = 512
    while j < ntot:
        nb = min(1024, ntot - j)
        subs = [SUB, nb - SUB] if nb > SUB else [nb]
        macros.append((j, nb, subs))
        j += nb

    ends = np.cumsum(c_sp)

    def segments(j0, nb):
        segs = []
        for sp in range(NUM_SPECIES):
            lo = int(ends[sp] - c_sp[sp])
            hi = int(ends[sp])
            a = max(lo, j0)
            b = min(hi, j0 + nb)
            if a < b:
                segs.append((sp, a - j0, b - a))
        return segs

    with tile.TileContext(nc) as tc:
        with ExitStack() as ctx:
            consts = ctx.enter_context(tc.tile_pool(name="consts", bufs=1))
            io_in = ctx.enter_context(tc.tile_pool(name="io_in", bufs=3))
            rhs_p = ctx.enter_context(tc.tile_pool(name="rhs", bufs=2))
            tmp = ctx.enter_context(tc.tile_pool(name="tmp", bufs=2))
            stag = ctx.enter_context(tc.tile_pool(name="stag", bufs=2))
            psum = ctx.enter_context(tc.tile_pool(name="psum", bufs=2, space="PSUM"))

            # --- constants (DMAs issued after the first input block below) ---
            w0_sb = consts.tile([128, 2, 256], bf16)
            w1_sb = consts.tile([128, 2, 256], bf16)
            a0_sb = consts.tile([128, 2, 5 * NUM_SPECIES], fp32)
            a1_sb = consts.tile([128, 2, 4 * NUM_SPECIES], fp32)
            consts_emitted = [False]

            def emit_const_dmas():
                nc.sync.dma_start(out=a0_sb, in_=a0[:].rearrange("(fc p) c -> p fc c", p=128))
                nc.sync.dma_start(out=a1_sb, in_=a1[:].rearrange("(fc p) c -> p fc c", p=128))
                nc.sync.dma_start(out=w0_sb, in_=w0[:].rearrange("(fc p) g -> p fc g", p=128))
                nc.sync.dma_start(out=w1_sb, in_=w1[:].rearrange("(fc p) g -> p fc g", p=128))
                consts_emitted[0] = True

            def a0c(fc, sp, p):
                i = sp * 5 + p
                return a0_sb[:, fc, i : i + 1]

            def a1c(fc, sp, p):
                i = sp * 4 + p
                return a1_sb[:, fc, i : i + 1]

            def flush(pend):
                """Deferred PSUM->SBUF copies for a finished sub-block.

                Emitted late (next sub-block or next macro) so the ACT queue
                sees upcoming affines before these PE-gated copies.
                """
                pnb, pairs, stg, so, _ymeta = pend
                for pi in range(2):
                    nc.scalar.activation(
                        stg[:, pi * 4 : pi * 4 + 4, so : so + pnb],
                        pairs[pi][:, :, :, :pnb].rearrange("p c g n -> p (c g) n"),
                        AF.Copy,
                    )

            pending = None
            for (j0, nb, subs) in macros:
                segs = segments(j0, nb)

                xin = io_in.tile([128, 5, 2, nb], bf16, tag="xin", name=f"xin_{j0}")
                nc.sync.dma_start(
                    out=xin.rearrange("p c f n -> p (c f) n"),
                    in_=xr[:, :, j0 : j0 + nb],
                )
                if not consts_emitted[0]:
                    emit_const_dmas()

                rhs = rhs_p.tile([128, 3, 2, nb], bf16, tag="rhs", name=f"rhs_{j0}")

                v_all = xin[:, 0:3, :, :]   # [128, 3, 2, nb]
                s_all = xin[:, 3, :, :]     # [128, 2, nb]
                vv = xin[:, 4, :, :]        # [128, 2, nb] host-precomputed v.v
                sv_pair = xin[:, 3:5, :, :]  # [s, vv] adjacent channels

                hg = tmp.tile([128, 2, 2, nb], bf16, tag="hg", bufs=2)
                h2 = hg[:, 0]
                gg = hg[:, 1]
                hb = tmp.tile([128, 2, 2, nb], bf16, tag="hb", bufs=2)
                h4 = hb[:, 0]
                bb = hb[:, 1]
                w_ = tmp.tile([128, 2, nb], bf16, tag="w", bufs=2)
                hp = tmp.tile([128, 2, 2, nb], bf16, tag="hp", bufs=2)
                h3 = hp[:, 0]
                p1 = hp[:, 1]
                at = tmp.tile([128, 2, 2, nb], bf16, tag="at", bufs=2)
                c1 = tmp.tile([128, 2, nb], bf16, tag="c1", bufs=2)

                # per-species affines: h2/w0 on ScalarE; bb/gg/w1 as DVE tensor_scalar
                for fc in range(2):
                    for (sp, o, L) in segs:
                        sl = slice(o, o + L)
                        nc.vector.tensor_scalar(
                            bb[:, fc, sl], s_all[:, fc, sl],
                            a0c(fc, sp, 4), a0c(fc, sp, 2), OP.mult, OP.add,
                        )
                        nc.scalar.activation(h2[:, fc, sl], s_all[:, fc, sl], AF.Identity,
                                             bias=a0c(fc, sp, 1), scale=a0c(fc, sp, 3))
                        nc.vector.tensor_scalar(
                            gg[:, fc, sl], s_all[:, fc, sl],
                            a1c(fc, sp, 2), a1c(fc, sp, 1), OP.mult, OP.add,
                        )
                        if fc == 0:  # split w across ACT/DVE to balance the queues
                            nc.scalar.activation(w_[:, fc, sl], vv[:, fc, sl], AF.Identity,
                                                 bias=a1c(fc, sp, 0), scale=a1c(fc, sp, 3))
                        else:
                            nc.vector.tensor_scalar(
                                w_[:, fc, sl], vv[:, fc, sl],
                                a1c(fc, sp, 3), a1c(fc, sp, 0), OP.mult, OP.add,
                            )

                # h3 = s*h2 ; p1 = s*gg  (one op, s broadcast over the pair)
                s_b2 = bass.AP(
                    tensor=xin.tensor,
                    offset=s_all.offset,
                    ap=[s_all.ap[0], [0, 2], s_all.ap[1], s_all.ap[2]],
                )
                nc.vector.tensor_tensor(hp, s_b2, hg, OP.mult)

                # h4 = h3 + a00 (single-op TS, 4x)
                for fc in range(2):
                    for (sp, o, L) in segs:
                        sl = slice(o, o + L)
                        nc.vector.tensor_scalar(
                            h4[:, fc, sl], h3[:, fc, sl], a0c(fc, sp, 0), None, OP.add,
                        )

                # [A0, T] = [s, vv] * [h4, bb]  (one paired op)
                nc.vector.tensor_tensor(at, sv_pair, hb, OP.mult)

                # c1 = p1 + w
                nc.vector.tensor_tensor(c1, p1, w_, OP.add)

                # rhs_c = c1 * v_c  (c1 broadcast over the 3 components)
                c1b = bass.AP(
                    tensor=c1.tensor,
                    offset=c1.offset,
                    ap=[c1.ap[0], [0, 3], c1.ap[1], c1.ap[2]],
                )
                nc.vector.tensor_tensor(rhs, c1b, v_all, OP.mult)

                # --- GEMM: y[comp] = rhs[comp] @ W (K=256 over fc chunks) ---
                # One staging tile + one y DMA per macro; PSUM tiles hold
                # comp pairs (01, 23) per SUB sub-block. Copies are deferred
                # one sub-block; the full-SUB pair stride keeps each
                # [*, c, g, :] matmul dest bank-aligned even for a ragged nb.
                stg = stag.tile([128, 8, nb], bf16, tag="stg", name=f"stg_{j0}")
                so = 0
                for snb in subs:
                    if pending is not None:
                        flush(pending)
                        if pending[4] is not None:
                            pj0, pnb, pstg = pending[4]
                            nc.sync.dma_start(
                                out=yr[:, :, pj0 : pj0 + pnb], in_=pstg
                            )
                    pairs = []
                    for pi in range(2):
                        pairs.append(
                            psum.tile([128, 2, 2, SUB], fp32, tag="ps",
                                      name=f"ps{pi}_{j0}_{so}")
                        )
                    for comp in range(4):
                        pst = pairs[comp // 2]
                        w_sb = w0_sb if comp == 0 else w1_sb
                        for gc in range(2):
                            g0 = gc * 128
                            for fc in range(2):
                                lhsT = w_sb[:, fc, g0 : g0 + 128]
                                movings = (
                                    [at[:, 0, fc, so : so + snb],
                                     at[:, 1, fc, so : so + snb]]
                                    if comp == 0
                                    else [rhs[:, comp - 1, fc, so : so + snb]]
                                )
                                for mi, mv in enumerate(movings):
                                    nc.tensor.matmul(
                                        pst[:, comp % 2, gc, :snb],
                                        lhsT,
                                        mv,
                                        start=(fc == 0 and mi == 0),
                                        stop=(fc == 1 and mi == len(movings) - 1),
                                    )
                    last = so + snb == nb
                    pending = (snb, pairs, stg, so,
                               (j0, nb, stg) if last else None)
                    so += snb

            flush(pending)
            if pending[4] is not None:
                pj0, pnb, pstg = pending[4]
                nc.sync.dma_start(out=yr[:, :, pj0 : pj0 + pnb], in_=pstg)

    nc.compile()
    return nc


def _prepare(node_feats, node_specie, w0, w1, W0, W1):
    """Host-side: sort by species, shard, transpose, fold scale factors."""
    n = node_feats.shape[0]
    sp = np.asarray(node_specie).astype(np.int64)

    ids_by_sp = [np.nonzero(sp == s)[0] for s in range(NUM_SPECIES)]
    # near-equal contiguous chunks per core (max-min <= 1) minimize the
    # per-species padded length (= max over cores)
    chunks = [np.array_split(ids_by_sp[s], N_CORES) for s in range(NUM_SPECIES)]
    core_ids = [[chunks[s][c] for s in range(NUM_SPECIES)] for c in range(N_CORES)]
    # even segment lengths keep bf16 slices 4B-aligned on device
    c_sp = [
        (max(len(core_ids[c][s]) for c in range(N_CORES)) + 1) // 2 * 2
        for s in range(NUM_SPECIES)
    ]
    ntot = int(np.sum(c_sp))  # even; tail block may be ragged (< SUB)

    idx = np.zeros((N_CORES, ntot), dtype=np.int64)
    valid = np.zeros((N_CORES, ntot), dtype=bool)
    off = 0
    for s in range(NUM_SPECIES):
        L = c_sp[s]
        for c in range(N_CORES):
            ids = core_ids[c][s]
            k = len(ids)
            idx[c, off : off + k] = ids
            valid[c, off : off + k] = True
        off += L

    w0a = np.asarray(w0, np.float32).copy()
    w1a = np.asarray(w1, np.float32).copy()
    w0a[:, 2, :] *= INV_SQRT3
    w0a[:, 4, :] *= INV_SQRT3
    w1a[:, 3, :] *= INV_SQRT3
    a0_tab = np.ascontiguousarray(
        w0a.transpose(2, 0, 1).reshape(F, 5 * NUM_SPECIES)
    )
    a1_tab = np.ascontiguousarray(
        w1a.transpose(2, 0, 1).reshape(F, 4 * NUM_SPECIES)
    )
    W0s = (np.asarray(W0, np.float32) * INV_SQRT_F).astype(ml_dtypes.bfloat16)
    W1s = (np.asarray(W1, np.float32) * INV_SQRT_F).astype(ml_dtypes.bfloat16)

    nf = np.asarray(node_feats, np.float32)
    n_ = nf.shape[0]
    # channel order [v1, v2, v3, s, vv]: s/vv adjacent for the paired
    # [A0, T] = [s, vv] * [h4, bb] tensor_tensor on device
    xf = np.empty((5, F, n_), np.float32)
    xf[0:3] = nf[:, :, 1:4].transpose(2, 1, 0)
    xf[3] = nf[:, :, 0].T
    xf[4] = (nf[:, :, 1] ** 2 + nf[:, :, 2] ** 2 + nf[:, :, 3] ** 2).T
    xt = xf.astype(ml_dtypes.bfloat16)  # [5,256,n]
    xs = []
    for c in range(N_CORES):
        xc = xt[:, :, idx[c]]
        xs.append(np.ascontiguousarray(xc.reshape(10, 128, ntot)))

    return xs, idx, valid, tuple(c_sp), ntot, a0_tab, a1_tab, W0s, W1s


def kernel(node_feats, node_specie, w0, w1, W0, W1):
    from concourse.bass_utils import run_bass_kernel_spmd

    xs, idx, valid, c_sp, ntot, a0_tab, a1_tab, W0s, W1s = _prepare(
        node_feats, node_specie, w0, w1, W0, W1
    )

    key = (c_sp, ntot)
    if key not in _KERNEL_CACHE:
        _KERNEL_CACHE[key] = _build_bass(list(c_sp), ntot)
    nc = _KERNEL_CACHE[key]

    in_maps = [
        {"x": xs[c], "a0": a0_tab, "a1": a1_tab, "w0": W0s, "w1": W1s}
        for c in range(N_CORES)
    ]
    res = run_bass_kernel_spmd(nc, in_maps, core_ids=list(range(N_CORES)))

    n = node_feats.shape[0]
    out = np.empty((n, F, 4), dtype=np.float32)
    for c in range(N_CORES):
        yc = res.results[c]["y"].reshape(4, F, ntot).astype(np.float32)
        yt = np.ascontiguousarray(yc.transpose(2, 1, 0))
        m = valid[c]
        out[idx[c][m]] = yt[m]
    return out


# revision 25
# speedup vs baseline: 1.2702x; 1.0349x over previous
"""Bass/Trainium2 kernel for nn_EquivariantProductBasisBlock.

Math (per node n, feature f):
    s = x[n,f,0]; v = x[n,f,1:4]; vv = (v.v)/sqrt(3)
    out0 = a0[sp,0]*s + a0[sp,1]*s^2 + a0[sp,2]*vv + a0[sp,3]*s^3 + a0[sp,4]*(s*vv)
    c1   = a1[sp,0] + a1[sp,1]*s + a1[sp,2]*s^2 + a1[sp,3]*vv
    y0 = out0 @ W0 / 16 ;  y1_c = (c1*v_c) @ W1 / 16
    out = concat(y0, y1) over the lm axis.

Strategy: shard nodes over 8 cores. Host sorts nodes by species so the
species-dependent path weights become per-partition scalar columns on
device (features on partitions, nodes on the free axis). The 1/sqrt(3)
and 1/16 factors are folded into the weight tables on the host.
Elementwise + GEMM run in bf16 (fp32 PSUM accumulation); I/O is bf16
with the final output upcast on the host.

Engine assignment (v3): DVE is the critical path. GPSIMD elementwise
offload was tried and reverted: its SBUF port is shared with the DVE
and concurrent streaming inflated DVE op costs ~3-5x. Instead the
h4 = h3 + a00 / out0 = s*h4 pair is fused into one
scalar_tensor_tensor (A0 = (h3 + a00) * s) and the w affine joins h2
on the ScalarE to balance DVE vs ACT.

Polynomial factorization (per-species coefficients fold into
ScalarE affines, tensor_scalar per-partition scalars, or the STT
scalar operand):
    h2 = a3*s + a1           (ACT affine, per species)
    w  = a13*vv + a10        (ACT affine)
    B  = a4*s + a2           (DVE tensor_scalar)
    gg = a12*s + a11         (DVE tensor_scalar)
    [h3, p1] = s * [h2, gg]  (DVE pair tensor_tensor)
    A0 = (h3 + a0) * s       (DVE scalar_tensor_tensor)
    T  = vv * B              (DVE tensor_tensor)
    out0 = A0 + T            (PE PSUM accumulation, two movings)
    c1 = p1 + w              (DVE tensor_tensor)
    rhs_c = c1 * v_c         (DVE tensor_tensor, c1 broadcast x3)
"""

import numpy as np
from contextlib import ExitStack

import ml_dtypes

N_CORES = 8
F = 256
NUM_SPECIES = 10
NB = 512    # nodes per compute block
SUB = 512   # nodes per matmul / store sub-block (one PSUM bank fp32)
INV_SQRT3 = 1.0 / np.sqrt(3.0)
INV_SQRT_F = 1.0 / np.sqrt(256.0)

_KERNEL_CACHE = {}


def _build_bass(c_sp, ntot):
    """Build + compile the per-core Bass graph.

    c_sp: per-species padded segment length (same on every core), even.
    ntot: total padded nodes per core (multiple of SUB).
    """
    import concourse.bacc as bacc
    import concourse.bass as bass
    import concourse.mybir as mybir
    import concourse.tile as tile

    fp32 = mybir.dt.float32
    bf16 = mybir.dt.bfloat16
    AF = mybir.ActivationFunctionType
    OP = mybir.AluOpType

    nc = bacc.Bacc("TRN2", target_bir_lowering=False, debug=False)

    x = nc.dram_tensor("x", [10, 128, ntot], bf16, kind="ExternalInput")
    a0 = nc.dram_tensor("a0", [256, 5 * NUM_SPECIES], fp32, kind="ExternalInput")
    a1 = nc.dram_tensor("a1", [256, 4 * NUM_SPECIES], fp32, kind="ExternalInput")
    w0 = nc.dram_tensor("w0", [256, 256], bf16, kind="ExternalInput")
    w1 = nc.dram_tensor("w1", [256, 256], bf16, kind="ExternalInput")
    y = nc.dram_tensor("y", [8, 128, ntot], bf16, kind="ExternalOutput")

    xr = x[:].rearrange("s p n -> p s n")
    yr = y[:].rearrange("s p n -> p s n")

    # macro blocks (one x/y DMA each) of two compute sub-blocks
    # first macro is small for pipeline ramp-in
    macros = [(0, 512, [256, 256])]
    j = 512
    while j < ntot:
        nb = min(1024, ntot - j)
        subs = [SUB, nb - SUB] if nb > SUB else [nb]
        macros.append((j, nb, subs))
        j += nb

    ends = np.cumsum(c_sp)

    def segments(j0, nb):
        segs = []
        for sp in range(NUM_SPECIES):
            lo = int(ends[sp] - c_sp[sp])
            hi = int(ends[sp])
            a = max(lo, j0)
            b = min(hi, j0 + nb)
            if a < b:
                segs.append((sp, a - j0, b - a))
        return segs

    with tile.TileContext(nc) as tc:
        with ExitStack() as ctx:
            consts = ctx.enter_context(tc.tile_pool(name="consts", bufs=1))
            io_in = ctx.enter_context(tc.tile_pool(name="io_in", bufs=4))
            rhs_p = ctx.enter_context(tc.tile_pool(name="rhs", bufs=2))
            tmp = ctx.enter_context(tc.tile_pool(name="tmp", bufs=2))
            stag = ctx.enter_context(tc.tile_pool(name="stag", bufs=2))
            psum = ctx.enter_context(tc.tile_pool(name="psum", bufs=2, space="PSUM"))

            # --- constants (DMAs issued after the first input block below) ---
            w0_sb = consts.tile([128, 2, 256], bf16)
            w1_sb = consts.tile([128, 2, 256], bf16)
            a0_sb = consts.tile([128, 2, 5 * NUM_SPECIES], fp32)
            a1_sb = consts.tile([128, 2, 4 * NUM_SPECIES], fp32)
            consts_emitted = [False]

            def emit_const_dmas():
                nc.sync.dma_start(out=a0_sb, in_=a0[:].rearrange("(fc p) c -> p fc c", p=128))
                nc.sync.dma_start(out=a1_sb, in_=a1[:].rearrange("(fc p) c -> p fc c", p=128))
                nc.sync.dma_start(out=w0_sb, in_=w0[:].rearrange("(fc p) g -> p fc g", p=128))
                nc.sync.dma_start(out=w1_sb, in_=w1[:].rearrange("(fc p) g -> p fc g", p=128))
                consts_emitted[0] = True

            def a0c(fc, sp, p):
                i = sp * 5 + p
                return a0_sb[:, fc, i : i + 1]

            def a1c(fc, sp, p):
                i = sp * 4 + p
                return a1_sb[:, fc, i : i + 1]

            def flush(pend):
                """Deferred PSUM->SBUF copies for a finished sub-block.

                Emitted late (next sub-block or next macro) so the ACT queue
                sees upcoming affines before these PE-gated copies.
                """
                pnb, pairs, stg, so, _ymeta = pend
                for pi in range(2):
                    nc.scalar.activation(
                        stg[:, pi * 4 : pi * 4 + 4, so : so + pnb],
                        pairs[pi][:, :, :, :pnb].rearrange("p c g n -> p (c g) n"),
                        AF.Copy,
                    )

            # input DMAs run PREFETCH macros ahead so their Sync-queue
            # triggers are not gated behind output-DMA triggers (which wait
            # on PSUM copies) and the data is resident before the affines
            PREFETCH = 3
            xin_tiles = {}

            def emit_x_dma(k):
                kj0, knb, _ = macros[k]
                t = io_in.tile([128, 5, 2, knb], bf16, tag="xin", name=f"xin_{kj0}")
                nc.sync.dma_start(
                    out=t.rearrange("p c f n -> p (c f) n"),
                    in_=xr[:, :, kj0 : kj0 + knb],
                )
                xin_tiles[k] = t

            for k in range(min(PREFETCH, len(macros))):
                emit_x_dma(k)
                if not consts_emitted[0]:
                    emit_const_dmas()

            pending = None
            for mi_, (j0, nb, subs) in enumerate(macros):
                segs = segments(j0, nb)
                xin = xin_tiles.pop(mi_)
                if mi_ + PREFETCH < len(macros):
                    emit_x_dma(mi_ + PREFETCH)

                rhs = rhs_p.tile([128, 3, 2, nb], bf16, tag="rhs", name=f"rhs_{j0}")

                v_all = xin[:, 0:3, :, :]   # [128, 3, 2, nb]
                s_all = xin[:, 3, :, :]     # [128, 2, nb]
                vv = xin[:, 4, :, :]        # [128, 2, nb] host-precomputed v.v
                sv_pair = xin[:, 3:5, :, :]  # [s, vv] adjacent channels

                hg = tmp.tile([128, 2, 2, nb], bf16, tag="hg", bufs=2)
                h2 = hg[:, 0]
                gg = hg[:, 1]
                hb = tmp.tile([128, 2, 2, nb], bf16, tag="hb", bufs=2)
                h4 = hb[:, 0]
                bb = hb[:, 1]
                w_ = tmp.tile([128, 2, nb], bf16, tag="w", bufs=2)
                hp = tmp.tile([128, 2, 2, nb], bf16, tag="hp", bufs=2)
                h3 = hp[:, 0]
                p1 = hp[:, 1]
                at = tmp.tile([128, 2, 2, nb], bf16, tag="at", bufs=2)
                c1 = tmp.tile([128, 2, nb], bf16, tag="c1", bufs=2)

                # per-species affines: h2/w0 on ScalarE; bb/gg/w1 as DVE tensor_scalar
                for fc in range(2):
                    for (sp, o, L) in segs:
                        sl = slice(o, o + L)
                        nc.vector.tensor_scalar(
                            bb[:, fc, sl], s_all[:, fc, sl],
                            a0c(fc, sp, 4), a0c(fc, sp, 2), OP.mult, OP.add,
                        )
                        nc.scalar.activation(h2[:, fc, sl], s_all[:, fc, sl], AF.Identity,
                                             bias=a0c(fc, sp, 1), scale=a0c(fc, sp, 3))
                        nc.vector.tensor_scalar(
                            gg[:, fc, sl], s_all[:, fc, sl],
                            a1c(fc, sp, 2), a1c(fc, sp, 1), OP.mult, OP.add,
                        )
                        if fc == 0:  # split w across ACT/DVE to balance the queues
                            nc.scalar.activation(w_[:, fc, sl], vv[:, fc, sl], AF.Identity,
                                                 bias=a1c(fc, sp, 0), scale=a1c(fc, sp, 3))
                        else:
                            nc.vector.tensor_scalar(
                                w_[:, fc, sl], vv[:, fc, sl],
                                a1c(fc, sp, 3), a1c(fc, sp, 0), OP.mult, OP.add,
                            )

                # h3 = s*h2 ; p1 = s*gg  (one op, s broadcast over the pair)
                s_b2 = bass.AP(
                    tensor=xin.tensor,
                    offset=s_all.offset,
                    ap=[s_all.ap[0], [0, 2], s_all.ap[1], s_all.ap[2]],
                )
                nc.vector.tensor_tensor(hp, s_b2, hg, OP.mult)

                # h4 = h3 + a00 (single-op TS, 4x)
                for fc in range(2):
                    for (sp, o, L) in segs:
                        sl = slice(o, o + L)
                        nc.vector.tensor_scalar(
                            h4[:, fc, sl], h3[:, fc, sl], a0c(fc, sp, 0), None, OP.add,
                        )

                # [A0, T] = [s, vv] * [h4, bb]  (one paired op)
                nc.vector.tensor_tensor(at, sv_pair, hb, OP.mult)

                # c1 = p1 + w
                nc.vector.tensor_tensor(c1, p1, w_, OP.add)

                # rhs_c = c1 * v_c  (c1 broadcast over the 3 components)
                c1b = bass.AP(
                    tensor=c1.tensor,
                    offset=c1.offset,
                    ap=[c1.ap[0], [0, 3], c1.ap[1], c1.ap[2]],
                )
                nc.vector.tensor_tensor(rhs, c1b, v_all, OP.mult)

                # --- GEMM: y[comp] = rhs[comp] @ W (K=256 over fc chunks) ---
                # One staging tile + one y DMA per macro; PSUM tiles hold
                # comp pairs (01, 23) per SUB sub-block. Copies are deferred
                # one sub-block; the full-SUB pair stride keeps each
                # [*, c, g, :] matmul dest bank-aligned even for a ragged nb.
                stg = stag.tile([128, 8, nb], bf16, tag="stg", name=f"stg_{j0}")
                so = 0
                for snb in subs:
                    if pending is not None:
                        flush(pending)
                        if pending[4] is not None:
                            pj0, pnb, pstg = pending[4]
                            nc.sync.dma_start(
                                out=yr[:, :, pj0 : pj0 + pnb], in_=pstg
                            )
                    pairs = []
                    for pi in range(2):
                        pairs.append(
                            psum.tile([128, 2, 2, SUB], fp32, tag="ps",
                                      name=f"ps{pi}_{j0}_{so}")
                        )
                    for comp in range(4):
                        pst = pairs[comp // 2]
                        w_sb = w0_sb if comp == 0 else w1_sb
                        for gc in range(2):
                            g0 = gc * 128
                            for fc in range(2):
                                lhsT = w_sb[:, fc, g0 : g0 + 128]
                                movings = (
                                    [at[:, 0, fc, so : so + snb],
                                     at[:, 1, fc, so : so + snb]]
                                    if comp == 0
                                    else [rhs[:, comp - 1, fc, so : so + snb]]
                                )
                                for mi, mv in enumerate(movings):
                                    nc.tensor.matmul(
                                        pst[:, comp % 2, gc, :snb],
                                        lhsT,
                                        mv,
                                        start=(fc == 0 and mi == 0),
                                        stop=(fc == 1 and mi == len(movings) - 1),
                                    )
                    last = so + snb == nb
                    pending = (snb, pairs, stg, so,
                               (j0, nb, stg) if last else None)
                    so += snb

            flush(pending)
            if pending[4] is not None:
                pj0, pnb, pstg = pending[4]
                nc.sync.dma_start(out=yr[:, :, pj0 : pj0 + pnb], in_=pstg)

    nc.compile()
    return nc


def _prepare(node_feats, node_specie, w0, w1, W0, W1):
    """Host-side: sort by species, shard, transpose, fold scale factors."""
    n = node_feats.shape[0]
    sp = np.asarray(node_specie).astype(np.int64)

    ids_by_sp = [np.nonzero(sp == s)[0] for s in range(NUM_SPECIES)]
    # near-equal contiguous chunks per core (max-min <= 1) minimize the
    # per-species padded length (= max over cores)
    chunks = [np.array_split(ids_by_sp[s], N_CORES) for s in range(NUM_SPECIES)]
    core_ids = [[chunks[s][c] for s in range(NUM_SPECIES)] for c in range(N_CORES)]
    # even segment lengths keep bf16 slices 4B-aligned on device
    c_sp = [
        (max(len(core_ids[c][s]) for c in range(N_CORES)) + 1) // 2 * 2
        for s in range(NUM_SPECIES)
    ]
    ntot = int(np.sum(c_sp))  # even; tail block may be ragged (< SUB)

    idx = np.zeros((N_CORES, ntot), dtype=np.int64)
    valid = np.zeros((N_CORES, ntot), dtype=bool)
    off = 0
    for s in range(NUM_SPECIES):
        L = c_sp[s]
        for c in range(N_CORES):
            ids = core_ids[c][s]
            k = len(ids)
            idx[c, off : off + k] = ids
            valid[c, off : off + k] = True
        off += L

    w0a = np.asarray(w0, np.float32).copy()
    w1a = np.asarray(w1, np.float32).copy()
    w0a[:, 2, :] *= INV_SQRT3
    w0a[:, 4, :] *= INV_SQRT3
    w1a[:, 3, :] *= INV_SQRT3
    a0_tab = np.ascontiguousarray(
        w0a.transpose(2, 0, 1).reshape(F, 5 * NUM_SPECIES)
    )
    a1_tab = np.ascontiguousarray(
        w1a.transpose(2, 0, 1).reshape(F, 4 * NUM_SPECIES)
    )
    W0s = (np.asarray(W0, np.float32) * INV_SQRT_F).astype(ml_dtypes.bfloat16)
    W1s = (np.asarray(W1, np.float32) * INV_SQRT_F).astype(ml_dtypes.bfloat16)

    nf = np.asarray(node_feats, np.float32)
    n_ = nf.shape[0]
    # channel order [v1, v2, v3, s, vv]: s/vv adjacent for the paired
    # [A0, T] = [s, vv] * [h4, bb] tensor_tensor on device
    xf = np.empty((5, F, n_), np.float32)
    xf[0:3] = nf[:, :, 1:4].transpose(2, 1, 0)
    xf[3] = nf[:, :, 0].T
    xf[4] = (nf[:, :, 1] ** 2 + nf[:, :, 2] ** 2 + nf[:, :, 3] ** 2).T
    xt = xf.astype(ml_dtypes.bfloat16)  # [5,256,n]
    xs = []
    for c in range(N_CORES):
        xc = xt[:, :, idx[c]]
        xs.append(np.ascontiguousarray(xc.reshape(10, 128, ntot)))

    return xs, idx, valid, tuple(c_sp), ntot, a0_tab, a1_tab, W0s, W1s


def kernel(node_feats, node_specie, w0, w1, W0, W1):
    from concourse.bass_utils import run_bass_kernel_spmd

    xs, idx, valid, c_sp, ntot, a0_tab, a1_tab, W0s, W1s = _prepare(
        node_feats, node_specie, w0, w1, W0, W1
    )

    key = (c_sp, ntot)
    if key not in _KERNEL_CACHE:
        _KERNEL_CACHE[key] = _build_bass(list(c_sp), ntot)
    nc = _KERNEL_CACHE[key]

    in_maps = [
        {"x": xs[c], "a0": a0_tab, "a1": a1_tab, "w0": W0s, "w1": W1s}
        for c in range(N_CORES)
    ]
    res = run_bass_kernel_spmd(nc, in_maps, core_ids=list(range(N_CORES)))

    n = node_feats.shape[0]
    out = np.empty((n, F, 4), dtype=np.float32)
    for c in range(N_CORES):
        yc = res.results[c]["y"].reshape(4, F, ntot).astype(np.float32)
        yt = np.ascontiguousarray(yc.transpose(2, 1, 0))
        m = valid[c]
        out[idx[c][m]] = yt[m]
    return out


# revision 30
# speedup vs baseline: 1.3523x; 1.0647x over previous
"""Bass/Trainium2 kernel for nn_EquivariantProductBasisBlock.

Math (per node n, feature f):
    s = x[n,f,0]; v = x[n,f,1:4]; vv = (v.v)/sqrt(3)
    out0 = a0[sp,0]*s + a0[sp,1]*s^2 + a0[sp,2]*vv + a0[sp,3]*s^3 + a0[sp,4]*(s*vv)
    c1   = a1[sp,0] + a1[sp,1]*s + a1[sp,2]*s^2 + a1[sp,3]*vv
    y0 = out0 @ W0 / 16 ;  y1_c = (c1*v_c) @ W1 / 16
    out = concat(y0, y1) over the lm axis.

Strategy: shard nodes over 8 cores. Host sorts nodes by species so the
species-dependent path weights become per-partition scalar columns on
device (features on partitions, nodes on the free axis). The 1/sqrt(3)
and 1/16 factors are folded into the weight tables on the host.
Elementwise + GEMM run in bf16 (fp32 PSUM accumulation); I/O is bf16
with the final output upcast on the host.

Engine assignment (v3): DVE is the critical path. GPSIMD elementwise
offload was tried and reverted: its SBUF port is shared with the DVE
and concurrent streaming inflated DVE op costs ~3-5x. Instead the
h4 = h3 + a00 / out0 = s*h4 pair is fused into one
scalar_tensor_tensor (A0 = (h3 + a00) * s) and the w affine joins h2
on the ScalarE to balance DVE vs ACT.

Polynomial factorization (per-species coefficients fold into
ScalarE affines, tensor_scalar per-partition scalars, or the STT
scalar operand):
    h2 = a3*s + a1           (ACT affine, per species)
    w  = a13*vv + a10        (ACT affine)
    B  = a4*s + a2           (DVE tensor_scalar)
    gg = a12*s + a11         (DVE tensor_scalar)
    [h3, p1] = s * [h2, gg]  (DVE pair tensor_tensor)
    A0 = (h3 + a0) * s       (DVE scalar_tensor_tensor)
    T  = vv * B              (DVE tensor_tensor)
    out0 = A0 + T            (PE PSUM accumulation, two movings)
    c1 = p1 + w              (DVE tensor_tensor)
    rhs_c = c1 * v_c         (DVE tensor_tensor, c1 broadcast x3)
"""

import numpy as np
from contextlib import ExitStack

import ml_dtypes

N_CORES = 8
F = 256
NUM_SPECIES = 10
NB = 512    # nodes per compute block
SUB = 512   # nodes per matmul / store sub-block (one PSUM bank fp32)
INV_SQRT3 = 1.0 / np.sqrt(3.0)
INV_SQRT_F = 1.0 / np.sqrt(256.0)

_KERNEL_CACHE = {}


def _build_bass(c_sp, ntot):
    """Build + compile the per-core Bass graph.

    c_sp: per-species padded segment length (same on every core), even.
    ntot: total padded nodes per core (multiple of SUB).
    """
    import concourse.bacc as bacc
    import concourse.bass as bass
    import concourse.mybir as mybir
    import concourse.tile as tile

    fp32 = mybir.dt.float32
    bf16 = mybir.dt.bfloat16
    AF = mybir.ActivationFunctionType
    OP = mybir.AluOpType

    nc = bacc.Bacc("TRN2", target_bir_lowering=False, debug=False)

    x = nc.dram_tensor("x", [10, 128, ntot], bf16, kind="ExternalInput")
    a0 = nc.dram_tensor("a0", [256, 5 * NUM_SPECIES], fp32, kind="ExternalInput")
    a1 = nc.dram_tensor("a1", [256, 4 * NUM_SPECIES], fp32, kind="ExternalInput")
    w0 = nc.dram_tensor("w0", [256, 256], bf16, kind="ExternalInput")
    w1 = nc.dram_tensor("w1", [256, 256], bf16, kind="ExternalInput")
    y = nc.dram_tensor("y", [8, 128, ntot], bf16, kind="ExternalOutput")

    xr = x[:].rearrange("s p n -> p s n")
    yr = y[:].rearrange("s p n -> p s n")

    # macro blocks (one x/y DMA each) of two compute sub-blocks
    # first macro is small for pipeline ramp-in
    macros = [(0, 512, [256, 256])]
    j = 512
    while j < ntot:
        nb = min(1024, ntot - j)
        subs = [SUB, nb - SUB] if nb > SUB else [nb]
        macros.append((j, nb, subs))
        j += nb

    ends = np.cumsum(c_sp)

    def segments(j0, nb):
        segs = []
        for sp in range(NUM_SPECIES):
            lo = int(ends[sp] - c_sp[sp])
            hi = int(ends[sp])
            a = max(lo, j0)
            b = min(hi, j0 + nb)
            if a < b:
                segs.append((sp, a - j0, b - a))
        return segs

    with tile.TileContext(nc) as tc:
        with ExitStack() as ctx:
            consts = ctx.enter_context(tc.tile_pool(name="consts", bufs=1))
            io_in = ctx.enter_context(tc.tile_pool(name="io_in", bufs=3))
            rhs_p = ctx.enter_context(tc.tile_pool(name="rhs", bufs=2))
            tmp = ctx.enter_context(tc.tile_pool(name="tmp", bufs=2))
            stag = ctx.enter_context(tc.tile_pool(name="stag", bufs=2))
            psum = ctx.enter_context(tc.tile_pool(name="psum", bufs=2, space="PSUM"))

            # --- constants (DMAs issued after the first input block below) ---
            w0_sb = consts.tile([128, 2, 256], bf16)
            w1_sb = consts.tile([128, 2, 256], bf16)
            a0_sb = consts.tile([128, 2, 5 * NUM_SPECIES], fp32)
            a1_sb = consts.tile([128, 2, 4 * NUM_SPECIES], fp32)
            consts_emitted = [False]

            def emit_const_dmas():
                nc.sync.dma_start(out=a0_sb, in_=a0[:].rearrange("(fc p) c -> p fc c", p=128))
                nc.sync.dma_start(out=a1_sb, in_=a1[:].rearrange("(fc p) c -> p fc c", p=128))
                nc.sync.dma_start(out=w0_sb, in_=w0[:].rearrange("(fc p) g -> p fc g", p=128))
                nc.sync.dma_start(out=w1_sb, in_=w1[:].rearrange("(fc p) g -> p fc g", p=128))
                consts_emitted[0] = True

            def a0c(fc, sp, p):
                i = sp * 5 + p
                return a0_sb[:, fc, i : i + 1]

            def a1c(fc, sp, p):
                i = sp * 4 + p
                return a1_sb[:, fc, i : i + 1]

            def flush(pend):
                """Deferred PSUM->SBUF copies + y DMA for a finished sub-block.

                Emitted late (next sub-block or next macro) so the ACT queue
                sees upcoming affines before these PE-gated copies.
                """
                pnb, pairs, pj0 = pend
                stg = stag.tile([128, 8, pnb], bf16, tag="stg", name=f"stg_{pj0}")
                for pi in range(2):
                    nc.scalar.activation(
                        stg[:, pi * 4 : pi * 4 + 4, :],
                        pairs[pi][:, :, :, :pnb].rearrange("p c g n -> p (c g) n"),
                        AF.Copy,
                    )
                nc.sync.dma_start(out=yr[:, :, pj0 : pj0 + pnb], in_=stg)

            # input DMAs run PREFETCH macros ahead so their Sync-queue
            # triggers are not gated behind output-DMA triggers (which wait
            # on PSUM copies) and the data is resident before the affines
            PREFETCH = 2
            xin_tiles = {}

            def emit_x_dma(k):
                kj0, knb, _ = macros[k]
                t = io_in.tile([128, 5, 2, knb], bf16, tag="xin", name=f"xin_{kj0}")
                nc.sync.dma_start(
                    out=t.rearrange("p c f n -> p (c f) n"),
                    in_=xr[:, :, kj0 : kj0 + knb],
                )
                xin_tiles[k] = t

            for k in range(min(PREFETCH, len(macros))):
                emit_x_dma(k)
                if not consts_emitted[0]:
                    emit_const_dmas()

            pending = None
            for mi_, (j0, nb, subs) in enumerate(macros):
                segs = segments(j0, nb)
                xin = xin_tiles.pop(mi_)
                if mi_ + PREFETCH < len(macros):
                    emit_x_dma(mi_ + PREFETCH)

                rhs = rhs_p.tile([128, 3, 2, nb], bf16, tag="rhs", name=f"rhs_{j0}")

                v_all = xin[:, 0:3, :, :]   # [128, 3, 2, nb]
                s_all = xin[:, 3, :, :]     # [128, 2, nb]
                vv = xin[:, 4, :, :]        # [128, 2, nb] host-precomputed v.v
                sv_pair = xin[:, 3:5, :, :]  # [s, vv] adjacent channels

                hg = tmp.tile([128, 2, 2, nb], bf16, tag="hg", bufs=2)
                h2 = hg[:, 0]
                gg = hg[:, 1]
                hb = tmp.tile([128, 2, 2, nb], bf16, tag="hb", bufs=2)
                h4 = hb[:, 0]
                bb = hb[:, 1]
                w_ = tmp.tile([128, 2, nb], bf16, tag="w", bufs=2)
                hp = tmp.tile([128, 2, 2, nb], bf16, tag="hp", bufs=2)
                h3 = hp[:, 0]
                p1 = hp[:, 1]
                at = tmp.tile([128, 2, 2, nb], bf16, tag="at", bufs=2)
                c1 = tmp.tile([128, 2, nb], bf16, tag="c1", bufs=2)

                # per-species affines: h2/w0 on ScalarE; bb/gg/w1 as DVE tensor_scalar
                for fc in range(2):
                    for (sp, o, L) in segs:
                        sl = slice(o, o + L)
                        nc.vector.tensor_scalar(
                            bb[:, fc, sl], s_all[:, fc, sl],
                            a0c(fc, sp, 4), a0c(fc, sp, 2), OP.mult, OP.add,
                        )
                        nc.scalar.activation(h2[:, fc, sl], s_all[:, fc, sl], AF.Identity,
                                             bias=a0c(fc, sp, 1), scale=a0c(fc, sp, 3))
                        nc.vector.tensor_scalar(
                            gg[:, fc, sl], s_all[:, fc, sl],
                            a1c(fc, sp, 2), a1c(fc, sp, 1), OP.mult, OP.add,
                        )
                        if fc == 0:  # split w across ACT/DVE to balance the queues
                            nc.scalar.activation(w_[:, fc, sl], vv[:, fc, sl], AF.Identity,
                                                 bias=a1c(fc, sp, 0), scale=a1c(fc, sp, 3))
                        else:
                            nc.vector.tensor_scalar(
                                w_[:, fc, sl], vv[:, fc, sl],
                                a1c(fc, sp, 3), a1c(fc, sp, 0), OP.mult, OP.add,
                            )

                # h3 = s*h2 ; p1 = s*gg  (one op, s broadcast over the pair)
                s_b2 = bass.AP(
                    tensor=xin.tensor,
                    offset=s_all.offset,
                    ap=[s_all.ap[0], [0, 2], s_all.ap[1], s_all.ap[2]],
                )
                nc.vector.tensor_tensor(hp, s_b2, hg, OP.mult)

                # h4 = h3 + a00 (single-op TS, 4x)
                for fc in range(2):
                    for (sp, o, L) in segs:
                        sl = slice(o, o + L)
                        nc.vector.tensor_scalar(
                            h4[:, fc, sl], h3[:, fc, sl], a0c(fc, sp, 0), None, OP.add,
                        )

                # [A0, T] = [s, vv] * [h4, bb]  (one paired op)
                nc.vector.tensor_tensor(at, sv_pair, hb, OP.mult)

                # c1 = p1 + w
                nc.vector.tensor_tensor(c1, p1, w_, OP.add)

                # rhs_c = c1 * v_c  (c1 broadcast over the 3 components)
                c1b = bass.AP(
                    tensor=c1.tensor,
                    offset=c1.offset,
                    ap=[c1.ap[0], [0, 3], c1.ap[1], c1.ap[2]],
                )
                nc.vector.tensor_tensor(rhs, c1b, v_all, OP.mult)

                # --- GEMM: y[comp] = rhs[comp] @ W (K=256 over fc chunks) ---
                # PSUM tiles hold comp pairs (01, 23) per SUB sub-block.
                # Copies+store-DMA are deferred one sub-block; the full-SUB
                # pair stride keeps each [*, c, g, :] matmul dest
                # bank-aligned even for a ragged nb.
                so = 0
                for snb in subs:
                    if pending is not None:
                        flush(pending)
                    pairs = []
                    for pi in range(2):
                        pairs.append(
                            psum.tile([128, 2, 2, SUB], fp32, tag="ps",
                                      name=f"ps{pi}_{j0}_{so}")
                        )
                    for comp in range(4):
                        pst = pairs[comp // 2]
                        w_sb = w0_sb if comp == 0 else w1_sb
                        for gc in range(2):
                            g0 = gc * 128
                            for fc in range(2):
                                lhsT = w_sb[:, fc, g0 : g0 + 128]
                                movings = (
                                    [at[:, 0, fc, so : so + snb],
                                     at[:, 1, fc, so : so + snb]]
                                    if comp == 0
                                    else [rhs[:, comp - 1, fc, so : so + snb]]
                                )
                                for mi, mv in enumerate(movings):
                                    nc.tensor.matmul(
                                        pst[:, comp % 2, gc, :snb],
                                        lhsT,
                                        mv,
                                        start=(fc == 0 and mi == 0),
                                        stop=(fc == 1 and mi == len(movings) - 1),
                                    )
                    pending = (snb, pairs, j0 + so)
                    so += snb

            flush(pending)

    nc.compile()
    return nc


def _prepare(node_feats, node_specie, w0, w1, W0, W1):
    """Host-side: sort by species, shard, transpose, fold scale factors."""
    n = node_feats.shape[0]
    sp = np.asarray(node_specie).astype(np.int64)

    ids_by_sp = [np.nonzero(sp == s)[0] for s in range(NUM_SPECIES)]
    # near-equal contiguous chunks per core (max-min <= 1) minimize the
    # per-species padded length (= max over cores)
    chunks = [np.array_split(ids_by_sp[s], N_CORES) for s in range(NUM_SPECIES)]
    core_ids = [[chunks[s][c] for s in range(NUM_SPECIES)] for c in range(N_CORES)]
    # even segment lengths keep bf16 slices 4B-aligned on device
    c_sp = [
        (max(len(core_ids[c][s]) for c in range(N_CORES)) + 1) // 2 * 2
        for s in range(NUM_SPECIES)
    ]
    ntot = int(np.sum(c_sp))  # even; tail block may be ragged (< SUB)

    idx = np.zeros((N_CORES, ntot), dtype=np.int64)
    valid = np.zeros((N_CORES, ntot), dtype=bool)
    off = 0
    for s in range(NUM_SPECIES):
        L = c_sp[s]
        for c in range(N_CORES):
            ids = core_ids[c][s]
            k = len(ids)
            idx[c, off : off + k] = ids
            valid[c, off : off + k] = True
        off += L

    w0a = np.asarray(w0, np.float32).copy()
    w1a = np.asarray(w1, np.float32).copy()
    w0a[:, 2, :] *= INV_SQRT3
    w0a[:, 4, :] *= INV_SQRT3
    w1a[:, 3, :] *= INV_SQRT3
    a0_tab = np.ascontiguousarray(
        w0a.transpose(2, 0, 1).reshape(F, 5 * NUM_SPECIES)
    )
    a1_tab = np.ascontiguousarray(
        w1a.transpose(2, 0, 1).reshape(F, 4 * NUM_SPECIES)
    )
    W0s = (np.asarray(W0, np.float32) * INV_SQRT_F).astype(ml_dtypes.bfloat16)
    W1s = (np.asarray(W1, np.float32) * INV_SQRT_F).astype(ml_dtypes.bfloat16)

    nf = np.asarray(node_feats, np.float32)
    n_ = nf.shape[0]
    # channel order [v1, v2, v3, s, vv]: s/vv adjacent for the paired
    # [A0, T] = [s, vv] * [h4, bb] tensor_tensor on device
    xf = np.empty((5, F, n_), np.float32)
    xf[0:3] = nf[:, :, 1:4].transpose(2, 1, 0)
    xf[3] = nf[:, :, 0].T
    xf[4] = (nf[:, :, 1] ** 2 + nf[:, :, 2] ** 2 + nf[:, :, 3] ** 2).T
    xt = xf.astype(ml_dtypes.bfloat16)  # [5,256,n]
    xs = []
    for c in range(N_CORES):
        xc = xt[:, :, idx[c]]
        xs.append(np.ascontiguousarray(xc.reshape(10, 128, ntot)))

    return xs, idx, valid, tuple(c_sp), ntot, a0_tab, a1_tab, W0s, W1s


def kernel(node_feats, node_specie, w0, w1, W0, W1):
    from concourse.bass_utils import run_bass_kernel_spmd

    xs, idx, valid, c_sp, ntot, a0_tab, a1_tab, W0s, W1s = _prepare(
        node_feats, node_specie, w0, w1, W0, W1
    )

    key = (c_sp, ntot)
    if key not in _KERNEL_CACHE:
        _KERNEL_CACHE[key] = _build_bass(list(c_sp), ntot)
    nc = _KERNEL_CACHE[key]

    in_maps = [
        {"x": xs[c], "a0": a0_tab, "a1": a1_tab, "w0": W0s, "w1": W1s}
        for c in range(N_CORES)
    ]
    res = run_bass_kernel_spmd(nc, in_maps, core_ids=list(range(N_CORES)))

    n = node_feats.shape[0]
    out = np.empty((n, F, 4), dtype=np.float32)
    for c in range(N_CORES):
        yc = res.results[c]["y"].reshape(4, F, ntot).astype(np.float32)
        yt = np.ascontiguousarray(yc.transpose(2, 1, 0))
        m = valid[c]
        out[idx[c][m]] = yt[m]
    return out
